# revision 35
# baseline (speedup 1.0000x reference)
"""Multi-query attention (nn_Attention) Trainium2 Bass kernel, 8-core SPMD.

Reference computation (fp32):
    q = einsum('bnd,hde->bhne', x, Wq) * dh**-0.5
    k, v = split(x @ Wkv)                      # shared across heads (MQA)
    out = softmax(q @ k^T) @ v                 # per head
    out = concat_heads(out) @ Wout

Shapes: x [2,2048,1024], Wq [16,1024,64], Wkv [1024,128], Wout [1024,1024].

Sharding: core = b*4 + g handles batch b and heads [4g, 4g+4). Wout is split
along its input (inner) dim, so each core produces a partial [2048,1024]
output; the host sums the 4 partials per batch.

The matmul datapath is bf16 (weights and activations cast once on write;
all accumulation stays fp32 in PSUM) — measured end-to-end error ~5e-3
against the fp32 reference, well inside the 2e-2 budget.

Per-core pipeline (tuned against the perfetto trace; ~190.5us measured,
down from the ~200-240us session baseline):
  1. All inputs are HOST-pre-packed into the exact SBUF layout so every
     input DMA moves 2-8KB contiguous runs per partition. Group-0 x
     kt-pairs ride the ACT hwdge queue concurrently with the weight DMAs
     on the sync queue (first exp at ~21us instead of ~28us).
  2. k/v projections are merged for groups 1-3: one [Wv|Wk] 8-MM chain
     (wkv packed as [Wk|Wk|Wv|Wk]); the second kT2 half is duplicated by
     an SBUF->SBUF DMA on the idle gpsimd SWDGE queue. Group 0 keeps the
     [Wk|Wk] form so the first-exp critical path stays short.
  3. it=0 interleaves the two head-pairs' j-loop HALVES so the exp supply
     stays dense while the later kv/v/q units build; v transposes are
     emitted in 2-block slices so the first attn@v unblocks early.
  4. Steady-state cycle: attn@v of an earlier jt (skew queue, keep=3 so
     its v_aug weight load is never late), the row-tiled scores pair, one
     1024-elem exp, and at most one deferred work item. The softmax
     normalize is split into 4 pieces deferred into the NEXT pair's
     cycles; attn@v accumulators live in a 3-slot PSUM ring so a pair
     boundary never stalls the exp cadence. qt projections for the next
     i-window are two N=256 half-bursts on the pop-free cycles 9/15.
  5. Tail: the last pair's chunk-0 output projections pre-run DURING the
     normalize chain (keeps the PE busy, so HAM never re-throttles to
     1.2GHz); chunk-1 accumulates after the MULTs; PSUM->SBUF copies
     alternate DVE/ACT and the four [128,1024] output DMAs alternate the
     sync/ACT queues.
"""

import os

import numpy as np
import ml_dtypes

import concourse.mybir as mybir
import concourse.tile as tile
from concourse import bacc
from concourse.bass_utils import run_bass_kernel_spmd
from concourse.dve_ops import RECIP_APPROX_FAST_CONSTS, RECIPROCAL_APPROX_FAST
from concourse.masks import make_identity

DIM = 1024
DIM_HEAD = 64
HEADS = 16
SCALE = DIM_HEAD**-0.5
B = 2
N = 2048
N_CORES = 8
HEADS_PER_CORE = HEADS // 4  # 4 head-groups across cores

P = 128
KT = DIM // P  # 8 contraction tiles
NT = N // P  # 16 row tiles of 128
IT = N // 512  # 4 i-tiles of 512
PAIRS = HEADS_PER_CORE // 2  # 2 head pairs
INNER = HEADS_PER_CORE * DIM_HEAD  # 256 per-core inner dim
CHUNKS = INNER // P  # 2 chunks of the inner dim
WKV_COLS = 256  # [Wk | Wk | Wv | Wk]: cols 0:128 for g=0, 128:256 merged


def _build():
    f32 = mybir.dt.float32
    f32r = mybir.dt.float32r
    bf16 = mybir.dt.bfloat16
    Exp = mybir.ActivationFunctionType.Exp

    nc = bacc.Bacc("TRN2", target_bir_lowering=False, debug=False,
                   enable_asserts=False)

    # all inputs are pre-packed on the host into the exact SBUF layout so
    # every input DMA moves 2-8KB contiguous runs per partition
    xt_d = nc.dram_tensor("xt", [P, IT, KT, 512], bf16, kind="ExternalInput")
    wq_d = nc.dram_tensor("wq", [PAIRS, P, KT, P], bf16, kind="ExternalInput")
    wkv_d = nc.dram_tensor("wkv", [2, P, KT, P], bf16, kind="ExternalInput")
    wout_d = nc.dram_tensor("wout", [P, CHUNKS, DIM], bf16,
                            kind="ExternalInput")
    out_d = nc.dram_tensor("out", [N, DIM], f32, kind="ExternalOutput")

    with tile.TileContext(nc) as tc:
        with (
            tc.tile_pool(name="const", bufs=1) as const,
            tc.tile_pool(name="w", bufs=1) as w,
            tc.tile_pool(name="big", bufs=1) as big,
            tc.tile_pool(name="expp", bufs=6) as expp,
            tc.tile_pool(name="small", bufs=2) as small,
            tc.tile_pool(name="outp", bufs=8) as outp,
            tc.tile_pool(name="ps_small", bufs=1, space="PSUM") as ps_small,
            tc.tile_pool(name="ps_sim", bufs=2, space="PSUM") as ps_sim,
            tc.tile_pool(name="ps_acc", bufs=3, space="PSUM") as ps_acc,
        ):
            identity_f = const.tile([P, P], f32)
            make_identity(nc, identity_f[:])
            identity = const.tile([P, P], f32r)
            nc.vector.tensor_copy(identity[:], identity_f[:])

            xT = big.tile([P, IT, KT, 512], bf16)

            def xt_dma(g):
                nc.sync.dma_start(xT[:, g, :, :], xt_d[:, g, :, :])

            def xt0_dma(kt0):
                # group-0 kt-pair DMAs ride the ACT queue, concurrent with
                # the weight DMAs on the sync queue
                nc.scalar.dma_start(
                    xT[:, 0, kt0:kt0 + 2, :], xt_d[:, 0, kt0:kt0 + 2, :]
                )

            wkv_sb = w.tile([P, 2, KT, P], bf16)
            wq_sb = w.tile([P, PAIRS, KT, P], bf16)

            xt0_dma(0)
            nc.sync.dma_start(wkv_sb[:, 0, :, :], wkv_d[0])
            xt0_dma(2)
            nc.sync.dma_start(wq_sb[:, 0, :, :], wq_d[0])
            xt0_dma(4)
            nc.sync.dma_start(wkv_sb[:, 1, :, :], wkv_d[1])
            xt0_dma(6)
            nc.sync.dma_start(wq_sb[:, 1, :, :], wq_d[1])
            for g in range(1, IT):
                xt_dma(g)
            wout_sb = w.tile([P, CHUNKS, DIM], bf16)
            nc.sync.dma_start(wout_sb[:], wout_d[:])

            onescol = const.tile([P, 1], f32)
            nc.gpsimd.memset(onescol[:], 1.0)
            ones65f = const.tile([65, 64], f32)
            nc.gpsimd.memset(ones65f[:], 1.0)
            ones65 = const.tile([65, 64], f32r)
            nc.vector.tensor_copy(ones65[64:65, :], ones65f[64:65, :])

            kT2 = big.tile([P, N], bf16)  # [kT; kT] stacked halves
            vT = big.tile([64, N], f32r)
            v_aug = big.tile([P, NT, 65], bf16)
            nc.vector.tensor_copy(
                v_aug[:, :, 64:65], onescol[:, None, :].to_broadcast((P, NT, 1))
            )
            qT = big.tile([P, PAIRS, N], bf16)
            oTn = big.tile([P, CHUNKS, N], bf16)
            rc = RECIP_APPROX_FAST_CONSTS

            def qt_proj(p, it, half=None, pool=None):
                if half is None:
                    n0, n1 = 0, 512
                else:
                    n0, n1 = half * 256, half * 256 + 256
                isl = slice(it * 512 + n0, it * 512 + n1)
                if pool is not None:
                    simt = pool.tile([P, 2, 512], f32, tag="sim", name="psq_pro")
                    psq = simt[:, 0, n0:n1]
                else:
                    psq = ps_small.tile([P, n1 - n0], f32, tag="pss", name="psq")
                for kt in range(KT):
                    nc.tensor.matmul(
                        psq.opt(),
                        wq_sb[:, p, kt, :],
                        xT[:, it, kt, n0:n1],
                        start=(kt == 0),
                        stop=(kt == KT - 1),
                    )
                nc.vector.tensor_copy(qT[:, p, isl], psq.opt())

            def kv_unit(g):
                # g=0 only: [Wk|Wk] stationary gives both kT2 halves with one
                # partition-aligned copy — keeps the first-exp path short.
                isl = slice(g * 512, (g + 1) * 512)
                psk = ps_small.tile([P, 512], f32, tag="pss", name="psk")
                for kt in range(KT):
                    nc.tensor.matmul(
                        psk[:],
                        wkv_sb[:, 0, kt, :],
                        xT[:, g, kt, :],
                        start=(kt == 0),
                        stop=(kt == KT - 1),
                    )
                nc.vector.tensor_copy(kT2[:, isl], psk[:])

            def v_proj(g):
                # g=0 only: separate Wv chain writing vT rows 0:64
                isl = slice(g * 512, (g + 1) * 512)
                psv = ps_small.tile([64, 512], f32, tag="pss", name="psv")
                for kt in range(KT):
                    nc.tensor.matmul(
                        psv[:],
                        wkv_sb[:, 1, kt, 0:64],
                        xT[:, g, kt, :],
                        start=(kt == 0),
                        stop=(kt == KT - 1),
                    )
                nc.vector.tensor_copy(vT[0:64, isl], psv[:])

            def kvm_unit(g):
                # merged [Wv|Wk] chain (wkv cols 128:256): rows 0:64 = vT,
                # rows 64:128 = kT. One 8-MM chain instead of two. The lower
                # kT2 half is duplicated by an SBUF->SBUF DMA on the sync
                # queue (lands well before this group's scores need it).
                isl = slice(g * 512, (g + 1) * 512)
                psk = ps_small.tile([P, 512], f32, tag="pss", name="pskm")
                for kt in range(KT):
                    nc.tensor.matmul(
                        psk[:],
                        wkv_sb[:, 1, kt, :],
                        xT[:, g, kt, :],
                        start=(kt == 0),
                        stop=(kt == KT - 1),
                    )
                nc.vector.tensor_copy(vT[0:64, isl], psk[0:64, :])
                nc.vector.tensor_copy(kT2[64:128, isl], psk[64:128, :])
                nc.gpsimd.dma_start(kT2[0:64, isl], kT2[64:128, isl])

            def v_trans(g, half=None):
                # transpose vT -> v_aug j-blocks; half splits the unit in two
                # so the first attn@v unblocks after only 2 transposes
                s0, s1 = (0, 4) if half is None else (2 * half, 2 * half + 2)
                psvt = ps_small.tile([P, s1 - s0, 64], f32r, tag="pss",
                                     name="psvt")
                for s in range(s0, s1):
                    jt = g * 4 + s
                    nc.tensor.matmul(
                        psvt[:, s - s0, :],
                        vT[0:64, jt * P:(jt + 1) * P],
                        identity[0:64, 0:64],
                        is_transpose=True,
                        start=(s == s0),
                        stop=(s == s1 - 1),
                    )
                nc.vector.tensor_copy(
                    v_aug[:, g * 4 + s0:g * 4 + s1, 0:64], psvt[:]
                )

            pending = []  # deferred normalize pieces + outproj groups
            skewq = []  # deferred attn@v emitters

            def flush_skew(keep=0):
                while len(skewq) > keep:
                    skewq.pop(0)()

            def emit_jt(it, p, po, jt, keep=2, pop=True):
                # attn@v of an earlier jt runs BEFORE this jt's scores so
                # the exp cadence survives pair boundaries and the v_aug
                # weight load prefetches under the previous matmul.
                flush_skew(keep=keep)
                isl = slice(it * 512, (it + 1) * 512)
                jsl = slice(jt * P, (jt + 1) * P)
                pss = ps_sim.tile([P, 2, 512], f32, tag="sim")
                for h in range(2):
                    nc.tensor.matmul(
                        pss[:, h, :],
                        kT2[64 * h:64 * (h + 1), jsl],
                        qT[64 * h:64 * (h + 1), p, isl],
                        tile_position=(64 * h, 0),
                    )
                et = expp.tile([P, 2, 512], bf16, tag="exp")
                nc.scalar.activation(et[:], pss[:], Exp, scale=SCALE)

                def do_oT(po=po, jt=jt, et=et):
                    for h in range(2):
                        nc.tensor.matmul(
                            po[h][:],
                            v_aug[:, jt, :],
                            et[:, h, :],
                            start=(jt % 8 == 0),
                            stop=(jt % 8 == 7),
                        )

                skewq.append(do_oT)
                if pop and pending:
                    pending.pop(0)()

            def alloc_po(it, p, half):
                return [
                    ps_acc.tile(
                        [65, 512], f32, tag="po", name=f"po{h}_{p}_{it}_{half}"
                    )
                    for h in range(2)
                ]

            def flush_half(po, oh):
                for h in range(2):
                    nc.vector.tensor_copy(oh[h][:], po[h][:])

            def alloc_oh(it, p):
                return [
                    small.tile([65, 512], f32r, tag=f"oh{h}", name=f"oh{h}_{p}_{it}")
                    for h in range(2)
                ]

            def norm_tail(it, p, h, ou, inline=False):
                isl = slice(it * 512, (it + 1) * 512)
                # PE bcast via K=1 matmul (gpsimd partition_broadcast was
                # tried and produced NaN: it does not honor a base_partition
                # other than 0 on the input AP)
                psb = ps_small.tile([64, 512], f32, tag="pss", name="psb")
                nc.tensor.matmul(psb[:], ones65[64:65, :], ou[64:65, :])
                src = psb
                rbc = small.tile([64, 512], f32, tag="rbc")
                nc.vector._custom_dve(
                    RECIPROCAL_APPROX_FAST,
                    out=rbc[:],
                    in0=src[:],
                    s0=rc["s0"],
                    s1=rc["s1"],
                    imm2=rc["imm2"],
                )
                nc.vector.tensor_tensor(
                    oTn[64 * h:64 * (h + 1), p, isl],
                    ou[0:64, :],
                    rbc[:],
                    mybir.AluOpType.mult,
                )

            def queue_normalize(it, p, oh, po2):
                ous = [None, None]

                def mk_add(h):
                    def f():
                        ou = small.tile(
                            [65, 512], f32r, tag=f"ou{h}", name=f"ou{h}_{p}_{it}"
                        )
                        nc.vector.tensor_tensor(
                            ou[:], oh[h][:], po2[h][:], mybir.AluOpType.add
                        )
                        ous[h] = ou

                    return f

                def mk_tail(h):
                    def f():
                        norm_tail(it, p, h, ous[h])

                    return f

                pending.extend([mk_add(0), mk_add(1), mk_tail(0), mk_tail(1)])

            def outproj_group(itt, dh):
                dsl = slice(dh * 512, (dh + 1) * 512)
                pso = ps_small.tile([P, 512], f32, tag="pss", name="pso")
                for c in range(CHUNKS):
                    nc.tensor.matmul(
                        pso[:],
                        oTn[:, c, itt * P:(itt + 1) * P],
                        wout_sb[:, c, dsl],
                        start=(c == 0),
                        stop=(c == CHUNKS - 1),
                    )
                os_ = outp.tile([P, 512], f32, tag="os")
                nc.vector.tensor_copy(os_[:], pso[:])
                nc.sync.dma_start(out_d[itt * P:(itt + 1) * P, dsl], os_[:])

            def queue_outproj(it):
                for t in range(4):
                    for dh in range(2):
                        pending.append(
                            lambda itt=it * 4 + t, d=dh: outproj_group(itt, d)
                        )

            # ---- Prologue: it=0, pair (0,0), units woven per-jt ----
            kv_unit(0)
            qt_proj(0, 0, pool=ps_sim)
            a0 = alloc_po(0, 0, 0)
            emit_jt(0, 0, a0, 0, keep=3)
            kvm_unit(1)
            emit_jt(0, 0, a0, 1, keep=3)
            v_proj(0)
            emit_jt(0, 0, a0, 2, keep=3)
            v_trans(0, 0)
            emit_jt(0, 0, a0, 3, keep=3)
            v_trans(0, 1)
            emit_jt(0, 0, a0, 4, keep=2)
            kvm_unit(2)
            emit_jt(0, 0, a0, 5, keep=2)
            qt_proj(1, 0)
            emit_jt(0, 0, a0, 6, keep=2)
            v_trans(1, 0)
            emit_jt(0, 0, a0, 7, keep=2)
            v_trans(1, 1)
            # (0,1) first half — the two pairs' halves interleave so the
            # exp supply stays dense while the later kv units build
            b0 = alloc_po(0, 1, 0)
            emit_jt(0, 1, b0, 0)
            emit_jt(0, 1, b0, 1)
            qt_proj(0, 1, half=0)
            emit_jt(0, 1, b0, 2)  # pops attn@v (0,0) jt=7 -> a0 complete
            ah = alloc_oh(0, 0)
            flush_half(a0, ah)
            emit_jt(0, 1, b0, 3)
            qt_proj(0, 1, half=1)
            emit_jt(0, 1, b0, 4)
            v_trans(2, 0)
            emit_jt(0, 1, b0, 5)
            v_trans(2, 1)
            emit_jt(0, 1, b0, 6)
            kvm_unit(3)
            emit_jt(0, 1, b0, 7)
            # (0,0) second half
            a1 = alloc_po(0, 0, 1)
            emit_jt(0, 0, a1, 8)
            emit_jt(0, 0, a1, 9)
            emit_jt(0, 0, a1, 10)  # pops attn@v (0,1) jt=7 -> b0 complete
            bh = alloc_oh(0, 1)
            flush_half(b0, bh)
            emit_jt(0, 0, a1, 11)
            v_trans(3, 0)
            emit_jt(0, 0, a1, 12)
            v_trans(3, 1)
            emit_jt(0, 0, a1, 13)
            qt_proj(1, 1, half=0)
            emit_jt(0, 0, a1, 14)
            qt_proj(1, 1, half=1)
            emit_jt(0, 0, a1, 15)
            queue_normalize(0, 0, ah, a1)
            # (0,1) second half
            b1 = alloc_po(0, 1, 1)
            for jt in range(8, 16):
                emit_jt(0, 1, b1, jt, pop=(jt >= 10))
            queue_normalize(0, 1, bh, b1)
            queue_outproj(0)

            # ---- Remaining (it, p) j-loops ----
            seq = [(it, p) for it in range(1, IT) for p in range(PAIRS)]
            for it, p in seq:
                last_pair = (it, p) == (IT - 1, PAIRS - 1)
                po = alloc_po(it, p, 0)
                po2 = None
                oh = None
                for jt in range(16):
                    if jt == 8:
                        po2 = alloc_po(it, p, 1)
                    tgt = po if jt < 8 else po2
                    qt_cycle = it + 1 < IT and jt in (9, 15)
                    # keep=3: attn@v trails its exp by a full extra cycle so
                    # the v_aug weight load is never late
                    emit_jt(it, p, tgt, jt, keep=3,
                            pop=(jt >= 3 and not qt_cycle))
                    if jt == 11:  # attn@v jt=7 has been popped -> po complete
                        oh = alloc_oh(it, p)
                        flush_half(po, oh)
                    if it + 1 < IT:
                        if jt == 9:
                            qt_proj(p, it + 1, half=0)
                        elif jt == 15:
                            qt_proj(p, it + 1, half=1)
                if not last_pair:
                    queue_normalize(it, p, oh, po2)
                    if p == PAIRS - 1:
                        queue_outproj(it)
                else:
                    # ---- Tail: inline normalize + last outproj groups.
                    # The chunk-0 matmuls only need pair (3,0)'s (already
                    # normalized) oTn, so they run DURING the normalize chain
                    # — keeping the PE busy (no HAM re-throttle) and off the
                    # critical path. chunk-1 accumulates after the MULTs.
                    flush_skew()
                    while pending:
                        pending.pop(0)()
                    ous = []
                    for h in range(2):
                        ou = small.tile(
                            [65, 512], f32r, tag=f"ou{h}", name=f"out{h}_tail"
                        )
                        nc.vector.tensor_tensor(
                            ou[:], oh[h][:], po2[h][:], mybir.AluOpType.add
                        )
                        ous.append(ou)
                    tail_psos = []
                    for gidx in range(8):
                        if gidx < 4:
                            if gidx % 2 == 0:
                                cur = ps_sim.tile(
                                    [P, 2, 512], f32, tag="sim",
                                    name=f"tpso{gidx}"
                                )
                            tail_psos.append(cur[:, gidx % 2, :])
                        elif gidx < 7:
                            tail_psos.append(ps_acc.tile(
                                [P, 512], f32, tag="po", name=f"tpso{gidx}"
                            ))
                        else:
                            tail_psos.append(ps_small.tile(
                                [P, 512], f32, tag="pss", name=f"tpso{gidx}"
                            ))

                    def tail_c(gidx, c):
                        t, dh = divmod(gidx, 2)
                        itt = (IT - 1) * 4 + t
                        dsl = slice(dh * 512, (dh + 1) * 512)
                        nc.tensor.matmul(
                            tail_psos[gidx].opt(),
                            oTn[:, c, itt * P:(itt + 1) * P],
                            wout_sb[:, c, dsl],
                            start=(c == 0),
                            stop=(c == CHUNKS - 1),
                        )

                    tail_c(0, 0)
                    tail_c(1, 0)
                    norm_tail(it, p, 0, ous[0], inline=True)
                    tail_c(2, 0)
                    tail_c(3, 0)
                    tail_c(4, 0)
                    norm_tail(it, p, 1, ous[1], inline=True)
                    tail_c(5, 0)
                    tail_c(6, 0)
                    tail_c(7, 0)
                    for t in range(4):
                        itt = (IT - 1) * 4 + t
                        os2 = outp.tile([P, 2, 512], f32, tag="os",
                                        name=f"os2_{t}")
                        tail_c(2 * t, 1)
                        nc.vector.tensor_copy(
                            os2[:, 0, :], tail_psos[2 * t].opt()
                        )
                        tail_c(2 * t + 1, 1)
                        nc.scalar.copy(os2[:, 1, :], tail_psos[2 * t + 1].opt())
                        eng = nc.sync if t % 2 == 0 else nc.scalar
                        eng.dma_start(
                            out_d[itt * P:(itt + 1) * P, :], os2[:].opt()
                        )

    nc.compile()
    return nc


_NC = None


def _get_nc():
    global _NC
    if _NC is None:
        _NC = _build()
    return _NC


def _sbufpack(mat, inner):
    """[DIM, inner] weight slice -> [P, KT, inner] (the SBUF layout), so the
    DMA descriptors are contiguous multi-KB runs per partition."""
    return np.ascontiguousarray(
        mat.reshape(KT, P, inner).transpose(1, 0, 2)
    )


def _prep_in_maps(x, Wq, Wkv, Wout):
    in_maps = []
    bf = ml_dtypes.bfloat16
    wk = Wkv[:, 0:DIM_HEAD].astype(np.float32).astype(bf)
    wv = Wkv[:, DIM_HEAD:].astype(np.float32).astype(bf)
    # half 0 = [Wk|Wk] (g=0 unit), half 1 = [Wv|Wk] (merged units)
    wkv_packed = np.ascontiguousarray(np.stack([
        _sbufpack(np.concatenate([wk, wk], axis=1), P),
        _sbufpack(np.concatenate([wv, wk], axis=1), P),
    ]))
    for core in range(N_CORES):
        b, g = divmod(core, 4)
        h0 = g * HEADS_PER_CORE
        wq_full = (
            np.transpose(Wq[h0:h0 + HEADS_PER_CORE], (1, 0, 2))
            .reshape(DIM, INNER)
            .astype(np.float32)
            .astype(bf)
        )
        wq = np.ascontiguousarray(np.stack(
            [_sbufpack(wq_full[:, p * P:(p + 1) * P], P) for p in range(PAIRS)]
        ))
        wout = np.ascontiguousarray(
            Wout[h0 * DIM_HEAD:(h0 + HEADS_PER_CORE) * DIM_HEAD]
            .astype(np.float32)
            .astype(bf)
            .reshape(CHUNKS, P, DIM)
            .transpose(1, 0, 2)
        )
        xt = np.ascontiguousarray(
            x[b].astype(np.float32).astype(bf).T
            .reshape(KT, P, IT, 512)
            .transpose(1, 2, 0, 3)
        )
        in_maps.append(
            {"xt": xt, "wq": wq, "wkv": wkv_packed, "wout": wout}
        )
    return in_maps


def _ensure_hook_shim():
    """bass_utils imports antenv.axon_hooks when tracing is requested via
    env (BASS_TRACE); that module is absent on this image. Provide a no-op
    fallback so an inherited env var cannot break a plain run."""
    try:
        import antenv.axon_hooks  # noqa: F401
    except Exception:
        import sys
        import types

        m = types.ModuleType("antenv.axon_hooks")
        m.get_axon_ntff_profile_hook = lambda: None
        m.set_axon_ntff_profile_hook = lambda h: None
        sys.modules["antenv.axon_hooks"] = m


def run(inputs, trace=False):
    """Run on 8 cores; returns (full_output, BassKernelResults)."""
    _ensure_hook_shim()
    nc = _get_nc()
    in_maps = _prep_in_maps(
        np.asarray(inputs["x"]),
        np.asarray(inputs["Wq"]),
        np.asarray(inputs["Wkv"]),
        np.asarray(inputs["Wout"]),
    )
    res = run_bass_kernel_spmd(
        nc, in_maps, core_ids=list(range(N_CORES)), trace=trace
    )
    out = np.zeros((B, N, DIM), dtype=np.float32)
    for core in range(N_CORES):
        b = core // 4
        out[b] += res.results[core]["out"]
    return out, res


def kernel(**inputs) -> np.ndarray:
    out, _ = run(inputs, trace=bool(os.environ.get("BASS_KERNEL_TRACE")))
    return out
